# revision 41
# baseline (speedup 1.0000x reference)
"""Multi-head attention + residual + LayerNorm, 8-core SPMD Trainium2 kernel.

Reference computation (B=4, S=2048, H=1024, 16 heads x 64):
    q/k/v = hs @ W{q,k,v}.T + b{q,k,v}           (per-head reshape)
    probs  = softmax(q k^T / 8)
    ctx    = probs @ v
    attn   = ctx @ Wo.T + bo
    out    = LayerNorm(attn + hs) * gamma + beta

Sharding: 8 shards = (batch b, sequence half sb).  Each core owns 1024 query
rows of one batch but computes K/V over the batch's full 2048 keys
(duplicated on the 2 sequence-half cores -> zero inter-core communication).
The host supplies hidden states pre-transposed with the core's OWN half
first; key order is a per-core permutation, which softmax sums make
harmless, and Q reads columns 0:1024 so the program stays rank-agnostic.

On-core data layouts (bf16 matmul operands, fp32 accumulation):
    hsT  [h, s]   transposed hidden states (host-pretransposed, no PE work)
    kT/qT[d, s]   per head-pair tiles [128, S]; q pre-scaled by 1/8
    V    [s, 65*16] heads strided by 65 with a ones column -> softmax sums
                  come out of the ctx matmul as row 64 ("ones trick")
    sT   [k, q]   scores transposed; exp on ScalarE without max subtraction
    cT   [d, q]   normalized context (bf16) feeding the output projection
    ySB  [q, h]   bf16 SBUF accumulator for the output projection, built
                  incrementally per head-pair so the tail stays short
"""

import os

import numpy as np

import concourse.bass as bass
import concourse.mybir as mybir
import concourse.tile as tile
from concourse import bacc
from concourse.bass_utils import run_bass_kernel_spmd

F32 = mybir.dt.float32
BF16 = mybir.dt.bfloat16
AF = mybir.ActivationFunctionType
OP = mybir.AluOpType

B, S, H = 4, 2048, 1024
NH, HD = 16, 64
SH = S // 2          # own query rows per core
N_CORES = 8
EPS = 1e-12

HT = H // 128        # 8 contraction tiles
ST = S // 128        # 16 key tiles
QB = SH // 512       # 2 q chunks
SB = S // 512        # 4 key chunks
HP = NH // 2         # 8 head-pair tiles

_CACHED_NC = {}


def _emit(tc, ln_id):
    nc = tc.nc
    hs_q = nc.dram_tensor("hs_q", [SH, H], F32, kind="ExternalInput").ap()
    hsT_d = nc.dram_tensor("hsT", [H, S], BF16, kind="ExternalInput").ap()
    wqT = nc.dram_tensor("wqT", [H, H], BF16, kind="ExternalInput").ap()
    wkT = nc.dram_tensor("wkT", [H, H], BF16, kind="ExternalInput").ap()
    wvT = nc.dram_tensor("wvT", [H, H], BF16, kind="ExternalInput").ap()
    woT = nc.dram_tensor("woT", [H, H], BF16, kind="ExternalInput").ap()
    bq_d = nc.dram_tensor("bq", [H], F32, kind="ExternalInput").ap()
    bk_d = nc.dram_tensor("bk", [H], F32, kind="ExternalInput").ap()
    bv_d = nc.dram_tensor("bv", [H], BF16, kind="ExternalInput").ap()
    bo_d = nc.dram_tensor("bo", [H], BF16, kind="ExternalInput").ap()
    gam_d = nc.dram_tensor("ln_gamma", [H], F32, kind="ExternalInput").ap()
    bet_d = nc.dram_tensor("ln_beta", [H], F32, kind="ExternalInput").ap()
    out_d = nc.dram_tensor("out", [SH, H], F32, kind="ExternalOutput").ap()

    # ---------------- persistent tiles ----------------
    persist = tc.alloc_tile_pool(name="persist", bufs=1)
    kT = [persist.tile([128, S], BF16, name=f"kT{i}") for i in range(HP)]
    qT = [persist.tile([128, SH], BF16, name=f"qT{i}") for i in range(HP)]
    vS = [persist.tile([128, NH * (HD + 1)], BF16, name=f"vS{i}") for i in range(ST)]
    ySB = [persist.tile([128, H], BF16, name=f"ySB{i}") for i in range(SH // 128)]

    const_p = tc.alloc_tile_pool(name="const", bufs=1)
    eps_t = const_p.tile([128, 1], F32, name="eps_t")
    nc.vector.memset(eps_t, EPS)
    bqc = const_p.tile([128, HT], F32, name="bqc")
    nc.sync.dma_start(out=bqc, in_=bq_d.rearrange("(j p) -> p j", p=128))
    nc.scalar.mul(bqc, bqc, 0.125)
    bkc = const_p.tile([128, HT], F32, name="bkc")
    nc.sync.dma_start(out=bkc, in_=bk_d.rearrange("(j p) -> p j", p=128))
    bob = const_p.tile([128, H], F32, name="bob")
    nc.gpsimd.dma_start(out=bob,
                        in_=bo_d.rearrange("(o n) -> o n", o=1).partition_broadcast(128))
    # ---------------- pools (opened in LIFO-release order) -------------------
    mm_ps = tc.alloc_tile_pool(name="mmps", bufs=2, space="PSUM")
    sc_ps = tc.alloc_tile_pool(name="scps", bufs=2, space="PSUM")
    cx_ps = tc.alloc_tile_pool(name="cxps", bufs=1, space="PSUM")
    dram_pool = tc.alloc_tile_pool(name="drampool", bufs=1, space="DRAM")
    w_pool = tc.alloc_tile_pool(name="wpool", bufs=1)
    nrm_pool = tc.alloc_tile_pool(name="nrmpool", bufs=2)
    p_pool = tc.alloc_tile_pool(name="ppool", bufs=4)
    ct_pool = tc.alloc_tile_pool(name="ctpool", bufs=2)
    wv_pool = tc.alloc_tile_pool(name="wvpool", bufs=1)

    # weights first on the DMA queue, then hsT by column chunks so the first
    # K-projection matmul can start after ~3MB of loads
    def load_w(pool, dram, nm):
        ws = [pool.tile([128, H], BF16, name=f"{nm}{i}") for i in range(HT)]
        wt = dram.rearrange("(t p) n -> t p n", p=128)
        for i in range(HT):
            nc.sync.dma_start(out=ws[i], in_=wt[i])
        return ws

    wk_s = load_w(w_pool, wkT, "wk")
    hsT = [w_pool.tile([128, S], BF16, name=f"hsT{i}") for i in range(HT)]
    hsT_t = hsT_d.rearrange("(t p) n -> t p n", p=128)
    for sc in range(SB):
        for i in range(HT):
            nc.gpsimd.dma_start(out=hsT[i][:, sc * 512:(sc + 1) * 512],
                                in_=hsT_t[i][:, sc * 512:(sc + 1) * 512])
    wv_s = load_w(wv_pool, wvT, "wv")
    wq_s = load_w(w_pool, wqT, "wq")
    bvb = wv_pool.tile([128, H], BF16, name="bvb")
    nc.sync.dma_start(out=bvb,
                      in_=bv_d.rearrange("(o n) -> o n", o=1).partition_broadcast(128))

    def proj_k(hp):
        for sc in range(SB):
            ps = mm_ps.tile([128, 512], F32, name="mm", tag="mm")
            for kt in range(HT):
                nc.tensor.matmul(ps, wk_s[kt][:, hp * 128:(hp + 1) * 128],
                                 hsT[kt][:, sc * 512:(sc + 1) * 512],
                                 start=(kt == 0), stop=(kt == HT - 1))
            nc.vector.tensor_scalar(out=kT[hp][:, sc * 512:(sc + 1) * 512], in0=ps,
                                    scalar1=bkc[:, hp:hp + 1], scalar2=None,
                                    op0=OP.add)

    def proj_v(st):
        """V rows for key-tile st, strided head layout [64 d cols + ones]."""
        vv = vS[st].rearrange("p (h e) -> p h e", e=HD + 1)
        for dc in range(2):
            ps = mm_ps.tile([128, 512], F32, name="mm", tag="mm")
            for kt in range(HT):
                nc.tensor.matmul(ps, hsT[kt][:, st * 128:(st + 1) * 128],
                                 wv_s[kt][:, dc * 512:(dc + 1) * 512],
                                 start=(kt == 0), stop=(kt == HT - 1))
            nc.vector.tensor_tensor(
                out=vv[:, dc * 8:(dc + 1) * 8, 0:HD],
                in0=ps.rearrange("p (h e) -> p h e", e=HD),
                in1=bvb[:, dc * 512:(dc + 1) * 512].rearrange(
                    "p (h e) -> p h e", e=HD),
                op=OP.add)
        nc.vector.memset(vv[:, :, HD:HD + 1], 1.0)

    def proj_q(hp):
        for qc in range(QB):
            ps = mm_ps.tile([128, 512], F32, name="mm", tag="mm")
            for kt in range(HT):
                nc.tensor.matmul(ps, wq_s[kt][:, hp * 128:(hp + 1) * 128],
                                 hsT[kt][:, qc * 512:(qc + 1) * 512],
                                 start=(kt == 0), stop=(kt == HT - 1))
            nc.vector.tensor_scalar(out=qT[hp][:, qc * 512:(qc + 1) * 512], in0=ps,
                                    scalar1=0.125, scalar2=bqc[:, hp:hp + 1],
                                    op0=OP.mult, op1=OP.add)

    # K for head-pair 0 first, then V/Q needed by the first heads; the rest
    # of the projections are emitted between attention heads so the scheduler
    # stuffs them into exp-stall bubbles.
    proj_k(0)
    for st in range(ST // 2):
        proj_v(st)
    proj_q(0)

    # ---------------- attention ----------------------------------------------
    def attn_scores(h, kt):
        """scores -> exp for one (head, key-tile); returns the probs tile."""
        hp, hh = divmod(h, 2)
        drows = slice(hh * 64, hh * 64 + 64)
        sps = sc_ps.tile([128, SH], F32, name="sc", tag="sc")
        for qc in range(QB):
            nc.tensor.matmul(sps[:, qc * 512:(qc + 1) * 512],
                             kT[hp][drows, kt * 128:(kt + 1) * 128],
                             qT[hp][drows, qc * 512:(qc + 1) * 512],
                             start=True, stop=True)
        pt = p_pool.tile([128, SH], BF16, name="pt", tag="pt")
        nc.scalar.activation(pt, sps, AF.Exp)
        return pt

    def attn_ctx(h, ctx_ps, kt, pt):
        for qc in range(QB):
            nc.tensor.matmul(ctx_ps[:, qc * 512:(qc + 1) * 512],
                             vS[kt][:, h * (HD + 1):(h + 1) * (HD + 1)],
                             pt[:, qc * 512:(qc + 1) * 512],
                             start=(kt == 0), stop=(kt == ST - 1))

    def attn_end(h, ctx_ps, ct):
        """Normalize by softmax sums (row HD) and evict to ct bf16.

        The PSUM slot is freed by a plain copy; the [1,q] -> [HD,q] broadcast
        bounces through DRAM (0-stride partition APs are only legal on DRAM
        sources), then a fast approximate reciprocal + multiply."""
        hh = h % 2
        drows = slice(hh * 64, hh * 64 + 64)
        stages = []
        for qc in range(QB):
            stage = nrm_pool.tile([HD + 1, 512], F32, name="stage", tag="stage")
            nc.vector.tensor_copy(stage, ctx_ps[:, qc * 512:(qc + 1) * 512])
            stages.append(stage)
        for qc in range(QB):
            stage = stages[qc]
            rrow = dram_pool.tile([1, 512], F32, name="rrow", tag="rrow", bufs=4)
            nc.sync.dma_start(out=rrow, in_=stage[HD:HD + 1, :])
            recb = nrm_pool.tile([HD, 512], F32, name="recb", tag="recb")
            nc.sync.dma_start(out=recb, in_=rrow.partition_broadcast(HD))
            nc.vector.reciprocal_approx_fast(out=recb, in_=recb)
            nc.vector.tensor_tensor(out=ct[drows, qc * 512:(qc + 1) * 512],
                                    in0=stage[0:HD, :], in1=recb,
                                    op=OP.mult)

    def attn_head(h, ct, v_tail=False):
        """2-deep software pipeline: ctx lags scores by 2 key-tiles so the
        PE never sits behind an exp it is waiting on."""
        ctx = cx_ps.tile([HD + 1, SH], F32, name="cx", tag="cx")
        pts = {}
        for kt in range(ST):
            pts[kt] = attn_scores(h, kt)
            if v_tail and kt < ST // 2:
                proj_v(ST // 2 + kt)
            if kt >= 2:
                attn_ctx(h, ctx, kt - 2, pts.pop(kt - 2))
        for kt in (ST - 2, ST - 1):
            attn_ctx(h, ctx, kt, pts.pop(kt))
        attn_end(h, ctx, ct)

    def out_proj(hp, ct):
        """Accumulate ct^T @ Wo[hp-block] into the bf16 SBUF accumulator."""
        for blk in range(SH // 128):
            for ec in range(2):
                ps = mm_ps.tile([128, 512], F32, name="mm", tag="mm")
                nc.tensor.matmul(ps, ct[:, blk * 128:(blk + 1) * 128],
                                 wo_s[hp][:, ec * 512:(ec + 1) * 512],
                                 start=True, stop=True)
                ysl = ySB[blk][:, ec * 512:(ec + 1) * 512]
                with nc.allow_low_precision(reason="bf16 out-proj accumulator"):
                    if hp == 0:
                        nc.vector.tensor_copy(ysl, ps)
                    else:
                        nc.vector.tensor_tensor(out=ysl, in0=ysl, in1=ps,
                                                op=OP.add)

    last_ct = None
    wo_s = None
    for hp in range(HP):
        ct = ct_pool.tile([128, SH], BF16, name="ct", tag="ct")
        attn_head(2 * hp, ct, v_tail=(hp == 0))
        if hp == 0:
            # V weights done; their hole hosts the output-projection weights
            wv_pool.release()
            wo_pool = tc.alloc_tile_pool(name="wopool", bufs=1)
            wo_s = load_w(wo_pool, woT, "wo")
        # remaining projections interleave into this pair's exp bubbles
        if hp + 1 < HP:
            proj_k(hp + 1)
        attn_head(2 * hp + 1, ct)
        if hp + 1 < HP:
            proj_q(hp + 1)
        out_proj(hp, ct)
        last_ct = ct
    if os.environ.get("K_DEBUG_DUMP"):
        ktd = nc.dram_tensor("kT_dump", [HP * 128, S], BF16,
                             kind="ExternalOutput").ap()
        vtd = nc.dram_tensor("vS_dump", [ST * 128, NH * (HD + 1)], BF16,
                             kind="ExternalOutput").ap()
        qtd = nc.dram_tensor("qT_dump", [HP * 128, SH], BF16,
                             kind="ExternalOutput").ap()
        ysd = nc.dram_tensor("ySB_dump", [SH, H], F32,
                             kind="ExternalOutput").ap()
        for hp in range(HP):
            nc.sync.dma_start(out=ktd[hp * 128:(hp + 1) * 128, :], in_=kT[hp])
            nc.sync.dma_start(out=qtd[hp * 128:(hp + 1) * 128, :], in_=qT[hp])
        for st in range(ST):
            nc.sync.dma_start(out=vtd[st * 128:(st + 1) * 128, :], in_=vS[st])
        for blk in range(SH // 128):
            nc.sync.dma_start(out=ysd[blk * 128:(blk + 1) * 128, :],
                              in_=ySB[blk])
        ctd = nc.dram_tensor("ct7_dump", [128, SH], BF16,
                             kind="ExternalOutput").ap()
        nc.sync.dma_start(out=ctd, in_=last_ct)

    # LIFO pop of everything above the phase-D working set
    wo_pool.release()
    ct_pool.release()
    p_pool.release()
    nrm_pool.release()
    w_pool.release()

    # ---------------- phase D: residual + LayerNorm --------------------------
    d_pool = tc.alloc_tile_pool(name="dpool", bufs=3)
    if not ln_id:
        gam_b = d_pool.tile([128, H], F32, name="gam_b", bufs=1)
        nc.sync.dma_start(out=gam_b,
                          in_=gam_d.rearrange("(o n) -> o n", o=1).partition_broadcast(128))
        bet_b = d_pool.tile([128, H], F32, name="bet_b", bufs=1)
        nc.sync.dma_start(out=bet_b,
                          in_=bet_d.rearrange("(o n) -> o n", o=1).partition_broadcast(128))

    hs_rows = hs_q.rearrange("(t p) n -> t p n", p=128)
    out_rows = out_d.rearrange("(t p) n -> t p n", p=128)
    for blk in range(SH // 128):
        res = d_pool.tile([128, H], F32, name="res", tag="res")
        nc.sync.dma_start(out=res, in_=hs_rows[blk])
        nc.gpsimd.tensor_tensor(out=res, in0=res, in1=bob, op=OP.add)
        x = d_pool.tile([128, H], F32, name="x", tag="x")
        nc.gpsimd.tensor_tensor(out=x, in0=ySB[blk], in1=res, op=OP.add)
        stats = d_pool.tile([128, 2, 6], F32, name="stats", tag="stats")
        xg = x.rearrange("p (g n) -> p g n", g=2)
        for g in range(2):
            nc.vector.bn_stats(out=stats[:, g, :], in_=xg[:, g, :])
        mv = d_pool.tile([128, 2], F32, name="mv", tag="mv")
        nc.vector.bn_aggr(out=mv, in_=stats)
        rstd = d_pool.tile([128, 1], F32, name="rstd", tag="rstd")
        nc.scalar.activation(rstd, mv[:, 1:2], AF.Sqrt, bias=eps_t)
        nc.vector.reciprocal(rstd, rstd)
        nmu = d_pool.tile([128, 1], F32, name="nmu", tag="nmu")
        nc.vector.tensor_tensor(out=nmu, in0=mv[:, 0:1], in1=rstd, op=OP.mult)
        nc.vector.tensor_scalar_mul(nmu, nmu, -1.0)
        y = d_pool.tile([128, H], F32, name="y", tag="y")
        nc.vector.tensor_scalar(out=y, in0=x, scalar1=rstd, scalar2=nmu,
                                op0=OP.mult, op1=OP.add)
        if not ln_id:
            nc.vector.tensor_tensor(out=y, in0=y, in1=gam_b, op=OP.mult)
            nc.vector.tensor_tensor(out=y, in0=y, in1=bet_b, op=OP.add)
        nc.sync.dma_start(out=out_rows[blk], in_=y)

    for pool in (d_pool, dram_pool, cx_ps, sc_ps, mm_ps, const_p, persist):
        pool.release()


def build_nc(ln_id=True):
    if ln_id in _CACHED_NC:
        return _CACHED_NC[ln_id]
    nc = bacc.Bacc("TRN2", target_bir_lowering=False, debug=False,
                   num_devices=N_CORES)
    with tile.TileContext(nc) as tc:
        _emit(tc, ln_id)
    nc.compile()
    _CACHED_NC[ln_id] = nc
    return nc


def make_in_maps(inputs):
    import ml_dtypes
    hs = np.ascontiguousarray(np.asarray(inputs["hidden_states"], dtype=np.float32))
    wT = {k: np.ascontiguousarray(np.asarray(inputs[k], np.float32).T
                                  .astype(ml_dtypes.bfloat16))
          for k in ("Wq", "Wk", "Wv", "Wo")}
    com = {
        "wqT": wT["Wq"], "wkT": wT["Wk"], "wvT": wT["Wv"], "woT": wT["Wo"],
        "bq": np.asarray(inputs["bq"], np.float32),
        "bk": np.asarray(inputs["bk"], np.float32),
        "bv": np.asarray(inputs["bv"], np.float32).astype(ml_dtypes.bfloat16),
        "bo": np.asarray(inputs["bo"], np.float32).astype(ml_dtypes.bfloat16),
        "ln_gamma": np.asarray(inputs["ln_gamma"], np.float32),
        "ln_beta": np.asarray(inputs["ln_beta"], np.float32),
    }
    in_maps = []
    for c in range(N_CORES):
        b, sb = divmod(c, 2)
        own = hs[b, sb * SH:(sb + 1) * SH]
        other = hs[b, (1 - sb) * SH:(2 - sb) * SH]
        hsT = np.concatenate([own.T, other.T], axis=1).astype(ml_dtypes.bfloat16)
        in_maps.append({
            "hs_q": np.ascontiguousarray(own),
            "hsT": np.ascontiguousarray(hsT),
            **com,
        })
    return in_maps


def gather_out(results):
    out = np.empty((B, S, H), np.float32)
    for c in range(N_CORES):
        b, sb = divmod(c, 2)
        out[b, sb * SH:(sb + 1) * SH, :] = results[c]["out"]
    return out


def kernel(**inputs) -> np.ndarray:
    ln_id = (np.all(np.asarray(inputs["ln_gamma"]) == 1.0)
             and np.all(np.asarray(inputs["ln_beta"]) == 0.0))
    nc = build_nc(bool(ln_id))
    res = run_bass_kernel_spmd(nc, make_in_maps(inputs), list(range(N_CORES)))
    return gather_out(res.results)
